# revision 11
# baseline (speedup 1.0000x reference)
"""APPNP (MLP + K-hop personalized-propagation + log_softmax) on 8 TRN2 NeuronCores.

Strategy (graph/data parallel, per sharding hint):
- Nodes are sharded row-wise across the 8 cores with a greedy assignment that
  balances per-(source-window, dest-block) in-edge counts across cores (the
  schedule is common/SPMD, so per-bin max over cores sets the padding).
- W1/W2 are replicated.
- Per hop, each core processes the in-edges of its node shard:
  z rows are fetched from a replicated copy of z in HBM via gpsimd.dma_gather
  (edge-major, 256B rows), scaled by the gcn norm on DVE, and segment-summed
  per destination with one-hot matmuls on the TensorEngine (PSUM accumulate).
- Halo exchange: each core's z shard is split into 4 position-slices and the
  replicated z is stored slice-interleaved ([slice][core][pos]), so the
  exchange is 4 independent AllGathers per hop.  Slice j's AllGather fires as
  soon as the last dest block of slice j finishes its final (q=3) window, so
  the AllGather for window 0 of hop k+1 overlaps the tail of hop k and the
  next hop's gathers start with no inter-hop bubble.
- int16 gather indices limit one gather to a <=32768-row window of z; the
  slice-interleaved layout makes window q exactly slice q (<=25600 rows).
- Self-loops are not gathered; their contribution nsl[i]*z[i] is applied from
  the resident agg tile.
"""

import sys

sys.path.insert(0, "/opt/trn_rl_repo")

import numpy as np
import ml_dtypes


def kernel(x, edge_index, edge_weight, W1, b1, W2, b2):
    out, _ = appnp_trn(
        np.asarray(x, dtype=np.float32),
        np.asarray(edge_index, dtype=np.int32),
        np.asarray(edge_weight, dtype=np.float32),
        np.asarray(W1, dtype=np.float32),
        np.asarray(b1, dtype=np.float32),
        np.asarray(W2, dtype=np.float32),
        np.asarray(b2, dtype=np.float32),
        k_hops=10,
        alpha=0.1,
    )
    return out


def _slice_sizes(SHARD, nslices=4):
    """Per-core position-slice sizes, multiples of 128 (whole dest blocks)."""
    nblk = SHARD // 128
    base = nblk // nslices
    rem = nblk - base * nslices
    blks = [base + (1 if s < rem else 0) for s in range(nslices)]
    return [b * 128 for b in blks]


def _assign_cores(row_e, col_e, N, ncores, SHARD):
    """Assign nodes to (core, position).  Position comes from the in-degree
    order (strata of 8 similar-degree nodes); the greedy balances, per
    stratum, the per-source-window in-edge counts across cores.  The source
    window of a node is slice_of(pos), which is independent of the core
    assignment, so a single greedy pass suffices."""
    cnt = np.bincount(col_e, minlength=N)
    order = np.argsort(cnt, kind="stable")  # ascending in-degree
    nstrata = N // ncores
    strata = order.reshape(nstrata, ncores)
    pos_of = np.empty(N, np.int64)
    pos_of[order] = np.repeat(np.arange(nstrata), ncores)

    sz = _slice_sizes(SHARD)
    p_off = np.concatenate([[0], np.cumsum(sz)])
    slice_of_pos = np.searchsorted(p_off[1:], np.arange(SHARD), side="right")

    nq = len(sz)
    src_q = slice_of_pos[pos_of[row_e]]
    dq = np.zeros((N, nq), np.float64)
    np.add.at(dq, (col_e, src_q), 1.0)

    new_core = np.empty(N, np.int64)
    dqs = dq[strata]  # [nstrata, 8, nq]
    tot = dqs.sum(axis=2)
    ordd = np.argsort(-tot, axis=1)
    for b0 in range(0, nstrata, 128):
        running = np.zeros((ncores, nq))
        for rr in range(b0, min(b0 + 128, nstrata)):
            used = 0
            nodes = strata[rr]
            for k in ordd[rr]:
                n = nodes[k]
                best, bestv = -1, None
                cand = running + dq[n]
                mx = cand.max(axis=1) + 1e-3 * cand.sum(axis=1)
                for c in range(ncores):
                    if used >> c & 1:
                        continue
                    if best < 0 or mx[c] < bestv:
                        best, bestv = c, mx[c]
                new_core[n] = best
                used |= 1 << best
                running[best] += dq[n]
    return new_core, pos_of


def _host_prep(edge_index, edge_weight, N, ncores, alpha, SHARD, GB, cnt_per_core):
    """Build the permutation, shards and padded slot schedules."""
    NBLK = SHARD // 128

    sz = _slice_sizes(SHARD)  # per-core slice sizes
    nq = len(sz)
    p_off = np.concatenate([[0], np.cumsum(sz)])  # pos offsets within a core
    w_sz = [s * ncores for s in sz]  # window sizes in z-replica
    w_off = np.concatenate([[0], np.cumsum(w_sz)])
    assert max(w_sz) <= 32768

    # self-loops are NOT gathered: their contribution nsl[i]*z[i] is computed
    # on-chip from the resident agg tile.
    row_e = edge_index[0].astype(np.int64)
    col_e = edge_index[1].astype(np.int64)
    w_e = edge_weight.astype(np.float64)
    ar = np.arange(N, dtype=np.int64)
    deg = np.bincount(np.concatenate([col_e, ar]),
                      weights=np.concatenate([w_e, np.ones(N)]), minlength=N)
    dis = np.where(deg > 0, 1.0 / np.sqrt(deg), 0.0)
    norm = (dis[row_e] * w_e * dis[col_e] * (1.0 - alpha)).astype(np.float32)
    nsl = ((1.0 - alpha) * dis * dis).astype(np.float32)  # self-loop weight

    core_of, pos_of = _assign_cores(row_e, col_e, N, ncores, SHARD)
    node_of = np.full((ncores, SHARD), 0, np.int64)
    node_of[core_of, pos_of] = np.arange(N)

    slice_of_pos = np.searchsorted(p_off[1:], np.arange(SHARD), side="right")
    # z-replica row of a node: window base + core*slice_size + pos-within-slice
    sq_of = slice_of_pos[pos_of]
    zrow = w_off[sq_of] + core_of * np.array(sz)[sq_of] + (pos_of - p_off[sq_of])

    src_q = sq_of[row_e]
    src_rel = (zrow[row_e] - w_off[src_q]).astype(np.int64)
    dest_core = core_of[col_e]
    dest_pos = pos_of[col_e]

    per_core = []
    counts = np.zeros((ncores, nq * NBLK), np.int64)
    for c in range(ncores):
        m = dest_core == c
        dc = dest_pos[m]
        key = src_q[m] * NBLK + (dc >> 7)
        o = np.argsort(key, kind="stable")
        per_core.append(
            (
                key[o],
                src_rel[m][o],
                norm[m][o],
                (dc & 127).astype(np.float32)[o],
            )
        )
        counts[c] = np.bincount(key, minlength=nq * NBLK)

    # common cross-core schedule: groups (of 128 slots) per (window, block)
    gqb = -(-counts.max(axis=0) // 128)  # ceil
    gqb = gqb.reshape(nq, NBLK)
    # every (window, block) needs >=1 group: q=0 initializes the accumulator,
    # q=3 triggers the per-block zsh write / next-slf staging
    gqb = np.maximum(gqb, 1)
    # pad each window's slot count to a multiple of GB (gather batch)
    for q in range(nq):
        Lq = 128 * gqb[q].sum()
        pad = (-Lq) % GB
        gqb[q, NBLK - 1] += pad // 128
    gqb = gqb.reshape(-1)
    group_off = np.concatenate([[0], np.cumsum(128 * gqb)])
    S_tot = int(group_off[-1])
    gqb2 = gqb.reshape(nq, NBLK)
    Lq_arr = [int(128 * gqb2[q].sum()) for q in range(nq)]
    chunk_off = np.concatenate([[0], np.cumsum(Lq_arr)]).astype(np.int64)

    idx_w = np.zeros((ncores, 128, S_tot // 16), np.int16)
    nrm_t = np.zeros((ncores, 128, S_tot // 128), np.float32)
    lds_t = np.zeros((ncores, 128, S_tot // 128), np.float32)
    for c in range(ncores):
        ks, rels, nrms, ldss = per_core[c]
        first = np.concatenate([[0], np.cumsum(counts[c])[:-1]])
        rank = np.arange(ks.size) - first[ks]
        pos = group_off[ks] + rank
        rel16 = np.zeros(S_tot, np.int16)
        nrm_s = np.zeros(S_tot, np.float32)
        lds_s = np.zeros(S_tot, np.float32)
        rel16[pos] = rels.astype(np.int16)
        nrm_s[pos] = nrms
        lds_s[pos] = ldss
        for q in range(nq):
            a, b = int(chunk_off[q]), int(chunk_off[q + 1])
            idx_w[c, :, a // 16 : b // 16] = np.tile(
                rel16[a:b].reshape(-1, 16).T, (8, 1)
            )
            nrm_t[c, :, a // 128 : b // 128] = nrm_s[a:b].reshape(-1, 128).T
            lds_t[c, :, a // 128 : b // 128] = lds_s[a:b].reshape(-1, 128).T

    nsl_t = np.zeros((ncores, 128, NBLK), np.float32)
    for c in range(ncores):
        nodes_c = node_of[c, : cnt_per_core[c]]
        pos = np.arange(cnt_per_core[c])
        nsl_t[c, pos & 127, pos >> 7] = nsl[nodes_c]

    return dict(
        node_of=node_of,
        gqb=gqb2,
        chunk_off=chunk_off,
        S_tot=S_tot,
        idx_w=idx_w,
        nrm_t=nrm_t,
        lds_t=lds_t,
        nsl_t=nsl_t,
        sz=sz,
        p_off=p_off,
        w_sz=w_sz,
    )


def appnp_trn(
    x,
    edge_index,
    edge_weight,
    W1,
    b1,
    W2,
    b2,
    k_hops,
    alpha,
    GB=1024,
    ncores=8,
    trace=False,
    use_f32r=False,
    use_bf16=True,
    nqueues=4,
):
    from concourse import bass, bacc, tile, mybir, library_config
    from concourse.bass_utils import run_bass_kernel_spmd

    N, F = x.shape
    H = W1.shape[1]
    C = W2.shape[1]
    assert C == 64 and H <= 128 and GB % 128 == 0

    base = N // ncores
    cnt_per_core = np.full(ncores, base, np.int64)
    cnt_per_core[: N - base * ncores] += 1
    SHARD = -(-int(cnt_per_core.max()) // 128) * 128
    NBLK = SHARD // 128
    FP = -(-F // 128) * 128
    KCH = FP // 128
    NQ = 4

    prep = _host_prep(edge_index, edge_weight, N, ncores, alpha, SHARD, GB, cnt_per_core)
    gqb = prep["gqb"]
    chunk_off = prep["chunk_off"]
    S_tot = prep["S_tot"]
    sz = prep["sz"]          # per-core slice sizes (rows)
    p_off = prep["p_off"]    # per-core slice offsets (rows)
    w_sz = prep["w_sz"]      # z-replica window sizes (rows)
    GPB = GB // 128  # groups per gather batch

    blk_per_slice = [s // 128 for s in sz]
    blk_off = np.concatenate([[0], np.cumsum(blk_per_slice)])
    last_blk_of_slice = [int(blk_off[j + 1] - 1) for j in range(NQ)]
    slice_of_blk = np.searchsorted(blk_off[1:], np.arange(NBLK), side="right")

    # ---- per-core inputs ----
    in_maps = []
    W1p = np.zeros((FP, H), ml_dtypes.bfloat16)
    W1p[:F] = W1
    b1c = b1.reshape(H, 1).astype(np.float32)
    b2r = np.tile(b2.reshape(1, C), (128, 1)).astype(np.float32)
    iota = np.tile(np.arange(128, dtype=np.float32), (128, 1))
    for c in range(ncores):
        xT = np.zeros((FP, SHARD), np.float32)
        nodes_c = prep["node_of"][c, : cnt_per_core[c]]
        xT[:F, : cnt_per_core[c]] = x[nodes_c].T
        in_maps.append(
            {
                "xT": xT.astype(ml_dtypes.bfloat16),
                "W1": W1p,
                "b1": b1c,
                "W2": W2.astype(np.float32),
                "b2r": b2r,
                "iota": iota,
                "nsl": prep["nsl_t"][c],
                "gidx": prep["idx_w"][c],
                "gnrm": prep["nrm_t"][c].astype(ml_dtypes.bfloat16),
                "glds": prep["lds_t"][c],
            }
        )

    # ---- build the device program ----
    nc = bacc.Bacc(
        "TRN2", target_bir_lowering=False, debug=False, num_devices=ncores,
        num_swdge_queues=nqueues,
    )
    dt = mybir.dt
    xT_d = nc.dram_tensor("xT", [FP, SHARD], dt.bfloat16, kind="ExternalInput").ap()
    W1_d = nc.dram_tensor("W1", [FP, H], dt.bfloat16, kind="ExternalInput").ap()
    b1_d = nc.dram_tensor("b1", [H, 1], dt.float32, kind="ExternalInput").ap()
    W2_d = nc.dram_tensor("W2", [H, C], dt.float32, kind="ExternalInput").ap()
    b2_d = nc.dram_tensor("b2r", [128, C], dt.float32, kind="ExternalInput").ap()
    io_d = nc.dram_tensor("iota", [128, 128], dt.float32, kind="ExternalInput").ap()
    ns_d = nc.dram_tensor("nsl", [128, SHARD // 128], dt.float32, kind="ExternalInput").ap()
    gi_d = nc.dram_tensor("gidx", [128, S_tot // 16], dt.int16, kind="ExternalInput").ap()
    gn_d = nc.dram_tensor("gnrm", [128, S_tot // 128], dt.bfloat16, kind="ExternalInput").ap()
    gl_d = nc.dram_tensor("glds", [128, S_tot // 128], dt.float32, kind="ExternalInput").ap()
    out_d = nc.dram_tensor("out", [SHARD, C], dt.float32, kind="ExternalOutput").ap()

    rg = [list(range(ncores))]

    with tile.TileContext(nc) as tc:
        nc.gpsimd.load_library(library_config.mlp)
        with (
            tc.tile_pool(name="const", bufs=1) as cst,
            tc.tile_pool(name="resident", bufs=1) as res,
            tc.tile_pool(name="gw", bufs=8) as gw,
            tc.tile_pool(name="rw", bufs=6) as rw,
            tc.tile_pool(name="sm", bufs=2) as sm,
            tc.tile_pool(name="psg", bufs=6, space="PSUM") as psg,
            tc.tile_pool(name="dram", bufs=1, space="DRAM") as dram,
        ):
            # constants / resident data
            W1_t = cst.tile([128, KCH * H], dt.bfloat16)
            for k in range(KCH):
                nc.sync.dma_start(W1_t[:, k * H : (k + 1) * H], W1_d[k * 128 : (k + 1) * 128, :])
            W2_t = cst.tile([H, C], dt.float32)
            nc.sync.dma_start(W2_t[:], W2_d[:])
            b1_t = cst.tile([H, 1], dt.float32)
            nc.sync.dma_start(b1_t[:], b1_d[:])
            b2_t = cst.tile([128, C], dt.float32)
            nc.sync.dma_start(b2_t[:], b2_d[:])
            io_t = cst.tile([128, 128], dt.float32)
            nc.sync.dma_start(io_t[:], io_d[:])
            ns_t = cst.tile([128, NBLK], dt.float32)
            nc.sync.dma_start(ns_t[:], ns_d[:])
            gi_t = res.tile([128, S_tot // 16], dt.int16)
            nc.sync.dma_start(gi_t[:], gi_d[:])
            gn_t = res.tile([128, S_tot // 128], dt.bfloat16)
            nc.sync.dma_start(gn_t[:], gn_d[:])
            gl_t = res.tile([128, S_tot // 128], dt.float32)
            nc.sync.dma_start(gl_t[:], gl_d[:])
            h01 = res.tile([128, NBLK, C], dt.bfloat16)
            agg = res.tile([128, NBLK, C], dt.float32)
            slf = [res.tile([128, NBLK, C], dt.float32, name=f"slf{i}") for i in range(2)]

            zsh = dram.tile([SHARD, C], dt.float32)
            # z replica, per (hop, window) tiles: window q holds slice q of
            # every core ([core][pos-within-slice] order).
            zwin = [
                [
                    dram.tile([w_sz[q], C], dt.float32, addr_space="Shared",
                              name=f"zw{k}_{q}")
                    for q in range(NQ)
                ]
                for k in range(k_hops)
            ]

            def fire_allgather(k, q):
                nc.gpsimd.collective_compute(
                    "AllGather", mybir.AluOpType.bypass, replica_groups=rg,
                    ins=[zsh[p_off[q] : p_off[q] + sz[q], :].opt()],
                    outs=[zwin[k][q][:].opt()],
                )

            # ---- MLP: h = relu(x @ W1 + b1) @ W2 + b2 ----
            with (
                tc.tile_pool(name="mlp", bufs=3) as mlp,
                tc.tile_pool(name="ps1", bufs=1, space="PSUM") as ps1,
                tc.tile_pool(name="ps2", bufs=1, space="PSUM") as ps2,
            ):
              for b in range(NBLK):
                xt = mlp.tile([128, KCH, 128], dt.bfloat16)
                for k in range(KCH):
                    nc.sync.dma_start(
                        xt[:, k, :], xT_d[k * 128 : (k + 1) * 128, b * 128 : (b + 1) * 128]
                    )
                p1 = ps1.tile([H, 128], dt.float32, space="PSUM")
                for k in range(KCH):
                    lw = W1_t[:, k * H : (k + 1) * H]
                    rw_ = xt[:, k, :]
                    if use_f32r:
                        lw = lw.bitcast(dt.float32r)
                        rw_ = rw_.bitcast(dt.float32r)
                    nc.tensor.matmul(
                        p1[:], lhsT=lw, rhs=rw_,
                        start=(k == 0), stop=(k == KCH - 1),
                    )
                rT = mlp.tile([H, 128], dt.float32)
                nc.scalar.activation(rT[:], p1[:], mybir.ActivationFunctionType.Relu, bias=b1_t[:])
                p2 = ps2.tile([128, C], dt.float32, space="PSUM")
                lw2, rw2 = rT[:], W2_t[:]
                if use_f32r:
                    lw2 = lw2.bitcast(dt.float32r)
                    rw2 = rw2.bitcast(dt.float32r)
                nc.tensor.matmul(p2[:], lhsT=lw2, rhs=rw2, start=True, stop=True)
                h = mlp.tile([128, C], dt.float32)
                nc.vector.tensor_tensor(h[:], p2[:], b2_t[:], op=mybir.AluOpType.add)
                nc.scalar.mul(h01[:, b, :], h[:], alpha)
                nc.vector.tensor_scalar(slf[0][:, b, :], h[:], ns_t[:, b : b + 1], None, op0=mybir.AluOpType.mult)
                nc.vector.tensor_tensor(slf[0][:, b, :], slf[0][:, b, :], h01[:, b, :], op=mybir.AluOpType.add)
                nc.sync.dma_start(zsh[b * 128 : (b + 1) * 128, :], h[:])
                for j in range(NQ):
                    if b == last_blk_of_slice[j]:
                        fire_allgather(0, j)

            def log_softmax_block(b):
                zb = agg[:, b, :]
                mx = sm.tile([128, 1], dt.float32)
                nc.vector.tensor_reduce(mx[:], zb, axis=mybir.AxisListType.X, op=mybir.AluOpType.max)
                zc = sm.tile([128, C], dt.float32)
                nc.vector.tensor_scalar(zc[:], zb, mx[:], None, op0=mybir.AluOpType.subtract)
                e = sm.tile([128, C], dt.float32)
                nc.scalar.activation(e[:], zc[:], mybir.ActivationFunctionType.Exp)
                sx = sm.tile([128, 1], dt.float32)
                nc.vector.tensor_reduce(sx[:], e[:], axis=mybir.AxisListType.X, op=mybir.AluOpType.add)
                ls = sm.tile([128, 1], dt.float32)
                nc.scalar.activation(ls[:], sx[:], mybir.ActivationFunctionType.Ln)
                o = sm.tile([128, C], dt.float32)
                nc.vector.tensor_scalar(o[:], zc[:], ls[:], None, op0=mybir.AluOpType.subtract)
                nc.sync.dma_start(out_d[b * 128 : (b + 1) * 128, :], o[:])

            # ---- K propagation hops ----
            for k in range(k_hops):
                cur, nxt = slf[k % 2], slf[(k + 1) % 2]
                last_hop = k == k_hops - 1
                for q in range(NQ):
                    a0 = int(chunk_off[q])
                    Lq = int(chunk_off[q + 1]) - a0
                    nbat = Lq // GB
                    zsrc = zwin[k][q][:]
                    # block schedule for this window: group -> block
                    blocks = np.repeat(np.arange(NBLK), gqb[q])
                    pcur = None
                    for bi in range(nbat):
                        s0 = a0 + bi * GB  # global slot offset
                        G = gw.tile([128, GPB, C], dt.float32)
                        nc.gpsimd.dma_gather(
                            out_ap=G[:],
                            in_ap=zsrc,
                            idxs_ap=gi_t[:, s0 // 16 : (s0 + GB) // 16],
                            num_idxs=GB,
                            num_idxs_reg=GB,
                            elem_size=C,
                            queue_num=(s0 // GB) % nqueues,
                        )
                        mmdt = dt.bfloat16 if use_bf16 else dt.float32
                        G2 = gw.tile([128, GPB, C], mmdt, name="G2", tag="G2")
                        nc.vector.tensor_tensor(
                            G2[:],
                            G[:],
                            gn_t[:, s0 // 128 : (s0 + GB) // 128].unsqueeze(2).to_broadcast([128, GPB, C]),
                            op=mybir.AluOpType.mult,
                        )
                        R = rw.tile([128, GPB, 128], mmdt)
                        nc.vector.tensor_tensor(
                            R[:],
                            io_t[:].unsqueeze(1).to_broadcast([128, GPB, 128]),
                            gl_t[:, s0 // 128 : (s0 + GB) // 128].unsqueeze(2).to_broadcast([128, GPB, 128]),
                            op=mybir.AluOpType.is_equal,
                        )
                        for j in range(GPB):
                            gg = bi * GPB + j  # group index within window
                            b = int(blocks[gg])
                            first = gg == 0 or int(blocks[gg - 1]) != b
                            last = gg == len(blocks) - 1 or int(blocks[gg + 1]) != b
                            if first:
                                pcur = psg.tile([128, C], dt.float32, space="PSUM", name="pg", tag="pg")
                            nc.tensor.matmul(
                                pcur[:], lhsT=R[:, j, :], rhs=G2[:, j, :],
                                start=first, stop=last,
                            )
                            if last:
                                if q == 0:
                                    nc.vector.tensor_tensor(
                                        agg[:, b, :], pcur[:],
                                        cur[:, b, :], op=mybir.AluOpType.add,
                                    )
                                else:
                                    nc.vector.tensor_tensor(
                                        agg[:, b, :],
                                        agg[:, b, :],
                                        pcur[:], op=mybir.AluOpType.add,
                                    )
                                if q == NQ - 1:
                                    if not last_hop:
                                        # block b final for this hop: write
                                        # its z rows; slice AllGathers fire
                                        # early
                                        nc.sync.dma_start(
                                            zsh[b * 128 : (b + 1) * 128, :], agg[:, b, :]
                                        )
                                        for j2 in range(NQ):
                                            if b == last_blk_of_slice[j2]:
                                                fire_allgather(k + 1, j2)
                                    else:
                                        log_softmax_block(b)
                if not last_hop:
                    # stage the next hop's self/teleport term in bulk
                    nc.vector.tensor_tensor(
                        nxt[:], agg[:],
                        ns_t[:].unsqueeze(2).to_broadcast([128, NBLK, C]),
                        op=mybir.AluOpType.mult,
                    )
                    nc.vector.tensor_tensor(nxt[:], nxt[:], h01[:], op=mybir.AluOpType.add)


    nc.compile()
    res_ = run_bass_kernel_spmd(nc, in_maps, core_ids=list(range(ncores)), trace=trace)

    out = np.empty((N, C), np.float32)
    for c in range(ncores):
        nodes_c = prep["node_of"][c, : cnt_per_core[c]]
        out[nodes_c] = res_.results[c]["out"][: cnt_per_core[c]]
    return out, res_


# revision 12
# speedup vs baseline: 1.0692x; 1.0692x over previous
"""APPNP (MLP + K-hop personalized-propagation + log_softmax) on 8 TRN2 NeuronCores.

Strategy (graph/data parallel, per sharding hint):
- Nodes are sharded row-wise across the 8 cores with a greedy assignment that
  balances per-(source-window, dest-block) in-edge counts across cores (the
  schedule is common/SPMD, so per-bin max over cores sets the padding).
- W1/W2 are replicated.
- Per hop, each core processes the in-edges of its node shard:
  z rows are fetched from a replicated copy of z in HBM via gpsimd.dma_gather
  (edge-major, 256B rows), scaled by the gcn norm on DVE, and segment-summed
  per destination with one-hot matmuls on the TensorEngine (PSUM accumulate).
- Halo exchange: each core's z shard is split into 4 position-slices and the
  replicated z is stored slice-interleaved ([slice][core][pos]), so the
  exchange is 4 independent AllGathers per hop.  Slice j's AllGather fires as
  soon as the last dest block of slice j finishes its final (q=3) window, so
  the AllGather for window 0 of hop k+1 overlaps the tail of hop k and the
  next hop's gathers start with no inter-hop bubble.
- int16 gather indices limit one gather to a <=32768-row window of z; the
  slice-interleaved layout makes window q exactly slice q (<=25600 rows).
- Self-loops are not gathered; their contribution nsl[i]*z[i] is applied from
  the resident agg tile.
"""

import sys

sys.path.insert(0, "/opt/trn_rl_repo")

import numpy as np
import ml_dtypes


def kernel(x, edge_index, edge_weight, W1, b1, W2, b2):
    out, _ = appnp_trn(
        np.asarray(x, dtype=np.float32),
        np.asarray(edge_index, dtype=np.int32),
        np.asarray(edge_weight, dtype=np.float32),
        np.asarray(W1, dtype=np.float32),
        np.asarray(b1, dtype=np.float32),
        np.asarray(W2, dtype=np.float32),
        np.asarray(b2, dtype=np.float32),
        k_hops=10,
        alpha=0.1,
    )
    return out


def _slice_sizes(SHARD, nslices=4):
    """Per-core position-slice sizes, multiples of 128 (whole dest blocks)."""
    nblk = SHARD // 128
    base = nblk // nslices
    rem = nblk - base * nslices
    blks = [base + (1 if s < rem else 0) for s in range(nslices)]
    return [b * 128 for b in blks]


def _assign_cores(row_e, col_e, N, ncores, SHARD):
    """Assign nodes to (core, position).  Position comes from the in-degree
    order (strata of 8 similar-degree nodes); the greedy balances, per
    stratum, the per-source-window in-edge counts across cores.  The source
    window of a node is slice_of(pos), which is independent of the core
    assignment, so a single greedy pass suffices."""
    cnt = np.bincount(col_e, minlength=N)
    order = np.argsort(cnt, kind="stable")  # ascending in-degree
    nstrata = N // ncores
    strata = order.reshape(nstrata, ncores)
    pos_of = np.empty(N, np.int64)
    pos_of[order] = np.repeat(np.arange(nstrata), ncores)

    sz = _slice_sizes(SHARD)
    p_off = np.concatenate([[0], np.cumsum(sz)])
    slice_of_pos = np.searchsorted(p_off[1:], np.arange(SHARD), side="right")

    nq = len(sz)
    src_q = slice_of_pos[pos_of[row_e]]
    dq = np.zeros((N, nq), np.float64)
    np.add.at(dq, (col_e, src_q), 1.0)

    new_core = np.empty(N, np.int64)
    dqs = dq[strata]  # [nstrata, 8, nq]
    tot = dqs.sum(axis=2)
    ordd = np.argsort(-tot, axis=1)
    for b0 in range(0, nstrata, 128):
        running = np.zeros((ncores, nq))
        for rr in range(b0, min(b0 + 128, nstrata)):
            used = 0
            nodes = strata[rr]
            for k in ordd[rr]:
                n = nodes[k]
                best, bestv = -1, None
                cand = running + dq[n]
                mx = cand.max(axis=1) + 1e-3 * cand.sum(axis=1)
                for c in range(ncores):
                    if used >> c & 1:
                        continue
                    if best < 0 or mx[c] < bestv:
                        best, bestv = c, mx[c]
                new_core[n] = best
                used |= 1 << best
                running[best] += dq[n]
    return new_core, pos_of


def _host_prep(edge_index, edge_weight, N, ncores, alpha, SHARD, GB, cnt_per_core):
    """Build the permutation, shards and padded slot schedules."""
    NBLK = SHARD // 128

    sz = _slice_sizes(SHARD)  # per-core slice sizes
    nq = len(sz)
    p_off = np.concatenate([[0], np.cumsum(sz)])  # pos offsets within a core
    w_sz = [s * ncores for s in sz]  # window sizes in z-replica
    w_off = np.concatenate([[0], np.cumsum(w_sz)])
    assert max(w_sz) <= 32768

    # self-loops are NOT gathered: their contribution nsl[i]*z[i] is computed
    # on-chip from the resident agg tile.
    row_e = edge_index[0].astype(np.int64)
    col_e = edge_index[1].astype(np.int64)
    w_e = edge_weight.astype(np.float64)
    ar = np.arange(N, dtype=np.int64)
    deg = np.bincount(np.concatenate([col_e, ar]),
                      weights=np.concatenate([w_e, np.ones(N)]), minlength=N)
    dis = np.where(deg > 0, 1.0 / np.sqrt(deg), 0.0)
    norm = (dis[row_e] * w_e * dis[col_e] * (1.0 - alpha)).astype(np.float32)
    nsl = ((1.0 - alpha) * dis * dis).astype(np.float32)  # self-loop weight

    core_of, pos_of = _assign_cores(row_e, col_e, N, ncores, SHARD)
    node_of = np.full((ncores, SHARD), 0, np.int64)
    node_of[core_of, pos_of] = np.arange(N)

    slice_of_pos = np.searchsorted(p_off[1:], np.arange(SHARD), side="right")
    # z-replica row of a node: window base + core*slice_size + pos-within-slice
    sq_of = slice_of_pos[pos_of]
    zrow = w_off[sq_of] + core_of * np.array(sz)[sq_of] + (pos_of - p_off[sq_of])

    src_q = sq_of[row_e]
    src_rel = (zrow[row_e] - w_off[src_q]).astype(np.int64)
    dest_core = core_of[col_e]
    dest_pos = pos_of[col_e]

    per_core = []
    counts = np.zeros((ncores, nq * NBLK), np.int64)
    for c in range(ncores):
        m = dest_core == c
        dc = dest_pos[m]
        key = src_q[m] * NBLK + (dc >> 7)
        o = np.argsort(key, kind="stable")
        per_core.append(
            (
                key[o],
                src_rel[m][o],
                norm[m][o],
                (dc & 127).astype(np.float32)[o],
            )
        )
        counts[c] = np.bincount(key, minlength=nq * NBLK)

    # common cross-core schedule: groups (of 128 slots) per (window, block)
    gqb = -(-counts.max(axis=0) // 128)  # ceil
    gqb = gqb.reshape(nq, NBLK)
    # every (window, block) needs >=1 group: q=0 initializes the accumulator,
    # q=3 triggers the per-block zsh write / next-slf staging
    gqb = np.maximum(gqb, 1)
    # pad each window's slot count to a multiple of GB (gather batch)
    for q in range(nq):
        Lq = 128 * gqb[q].sum()
        pad = (-Lq) % GB
        gqb[q, NBLK - 1] += pad // 128
    gqb = gqb.reshape(-1)
    group_off = np.concatenate([[0], np.cumsum(128 * gqb)])
    S_tot = int(group_off[-1])
    gqb2 = gqb.reshape(nq, NBLK)
    Lq_arr = [int(128 * gqb2[q].sum()) for q in range(nq)]
    chunk_off = np.concatenate([[0], np.cumsum(Lq_arr)]).astype(np.int64)

    idx_w = np.zeros((ncores, 128, S_tot // 16), np.int16)
    nrm_t = np.zeros((ncores, 128, S_tot // 128), np.float32)
    lds_t = np.zeros((ncores, 128, S_tot // 128), np.float32)
    for c in range(ncores):
        ks, rels, nrms, ldss = per_core[c]
        first = np.concatenate([[0], np.cumsum(counts[c])[:-1]])
        rank = np.arange(ks.size) - first[ks]
        pos = group_off[ks] + rank
        rel16 = np.zeros(S_tot, np.int16)
        nrm_s = np.zeros(S_tot, np.float32)
        lds_s = np.zeros(S_tot, np.float32)
        rel16[pos] = rels.astype(np.int16)
        nrm_s[pos] = nrms
        lds_s[pos] = ldss
        for q in range(nq):
            a, b = int(chunk_off[q]), int(chunk_off[q + 1])
            idx_w[c, :, a // 16 : b // 16] = np.tile(
                rel16[a:b].reshape(-1, 16).T, (8, 1)
            )
            nrm_t[c, :, a // 128 : b // 128] = nrm_s[a:b].reshape(-1, 128).T
            lds_t[c, :, a // 128 : b // 128] = lds_s[a:b].reshape(-1, 128).T

    nsl_t = np.zeros((ncores, 128, NBLK), np.float32)
    for c in range(ncores):
        nodes_c = node_of[c, : cnt_per_core[c]]
        pos = np.arange(cnt_per_core[c])
        nsl_t[c, pos & 127, pos >> 7] = nsl[nodes_c]

    return dict(
        node_of=node_of,
        gqb=gqb2,
        chunk_off=chunk_off,
        S_tot=S_tot,
        idx_w=idx_w,
        nrm_t=nrm_t,
        lds_t=lds_t,
        nsl_t=nsl_t,
        sz=sz,
        p_off=p_off,
        w_sz=w_sz,
    )


def appnp_trn(
    x,
    edge_index,
    edge_weight,
    W1,
    b1,
    W2,
    b2,
    k_hops,
    alpha,
    GB=1024,
    ncores=8,
    trace=False,
    use_f32r=False,
    use_bf16=True,
    nqueues=4,
):
    from concourse import bass, bacc, tile, mybir, library_config
    from concourse.bass_utils import run_bass_kernel_spmd

    N, F = x.shape
    H = W1.shape[1]
    C = W2.shape[1]
    assert C == 64 and H <= 128 and GB % 128 == 0

    base = N // ncores
    cnt_per_core = np.full(ncores, base, np.int64)
    cnt_per_core[: N - base * ncores] += 1
    SHARD = -(-int(cnt_per_core.max()) // 128) * 128
    NBLK = SHARD // 128
    FP = -(-F // 128) * 128
    KCH = FP // 128
    NQ = 4

    prep = _host_prep(edge_index, edge_weight, N, ncores, alpha, SHARD, GB, cnt_per_core)
    gqb = prep["gqb"]
    chunk_off = prep["chunk_off"]
    S_tot = prep["S_tot"]
    sz = prep["sz"]          # per-core slice sizes (rows)
    p_off = prep["p_off"]    # per-core slice offsets (rows)
    w_sz = prep["w_sz"]      # z-replica window sizes (rows)
    GPB = GB // 128  # groups per gather batch

    blk_per_slice = [s // 128 for s in sz]
    blk_off = np.concatenate([[0], np.cumsum(blk_per_slice)])
    last_blk_of_slice = [int(blk_off[j + 1] - 1) for j in range(NQ)]
    slice_of_blk = np.searchsorted(blk_off[1:], np.arange(NBLK), side="right")

    # ---- per-core inputs ----
    in_maps = []
    W1p = np.zeros((FP, H), ml_dtypes.bfloat16)
    W1p[:F] = W1
    b1c = b1.reshape(H, 1).astype(np.float32)
    b2r = np.tile(b2.reshape(1, C), (128, 1)).astype(np.float32)
    iota = np.tile(np.arange(128, dtype=np.float32), (128, 1))
    for c in range(ncores):
        xT = np.zeros((FP, SHARD), np.float32)
        nodes_c = prep["node_of"][c, : cnt_per_core[c]]
        xT[:F, : cnt_per_core[c]] = x[nodes_c].T
        in_maps.append(
            {
                "xT": xT.astype(ml_dtypes.bfloat16),
                "W1": W1p,
                "b1": b1c,
                "W2": W2.astype(np.float32),
                "b2r": b2r,
                "iota": iota,
                "nsl": prep["nsl_t"][c],
                "gidx": prep["idx_w"][c],
                "gnrm": prep["nrm_t"][c].astype(ml_dtypes.bfloat16),
                "glds": prep["lds_t"][c],
            }
        )

    # ---- build the device program ----
    nc = bacc.Bacc(
        "TRN2", target_bir_lowering=False, debug=False, num_devices=ncores,
        num_swdge_queues=nqueues,
    )
    dt = mybir.dt
    xT_d = nc.dram_tensor("xT", [FP, SHARD], dt.bfloat16, kind="ExternalInput").ap()
    W1_d = nc.dram_tensor("W1", [FP, H], dt.bfloat16, kind="ExternalInput").ap()
    b1_d = nc.dram_tensor("b1", [H, 1], dt.float32, kind="ExternalInput").ap()
    W2_d = nc.dram_tensor("W2", [H, C], dt.float32, kind="ExternalInput").ap()
    b2_d = nc.dram_tensor("b2r", [128, C], dt.float32, kind="ExternalInput").ap()
    io_d = nc.dram_tensor("iota", [128, 128], dt.float32, kind="ExternalInput").ap()
    ns_d = nc.dram_tensor("nsl", [128, SHARD // 128], dt.float32, kind="ExternalInput").ap()
    gi_d = nc.dram_tensor("gidx", [128, S_tot // 16], dt.int16, kind="ExternalInput").ap()
    gn_d = nc.dram_tensor("gnrm", [128, S_tot // 128], dt.bfloat16, kind="ExternalInput").ap()
    gl_d = nc.dram_tensor("glds", [128, S_tot // 128], dt.float32, kind="ExternalInput").ap()
    out_d = nc.dram_tensor("out", [SHARD, C], dt.float32, kind="ExternalOutput").ap()

    rg = [list(range(ncores))]

    with tile.TileContext(nc) as tc:
        nc.gpsimd.load_library(library_config.mlp)
        with (
            tc.tile_pool(name="const", bufs=1) as cst,
            tc.tile_pool(name="resident", bufs=1) as res,
            tc.tile_pool(name="gw", bufs=8) as gw,
            tc.tile_pool(name="rw", bufs=6) as rw,
            tc.tile_pool(name="sm", bufs=2) as sm,
            tc.tile_pool(name="psg", bufs=6, space="PSUM") as psg,
            tc.tile_pool(name="dram", bufs=1, space="DRAM") as dram,
        ):
            # constants / resident data
            W1_t = cst.tile([128, KCH * H], dt.bfloat16)
            for k in range(KCH):
                nc.sync.dma_start(W1_t[:, k * H : (k + 1) * H], W1_d[k * 128 : (k + 1) * 128, :])
            W2_t = cst.tile([H, C], dt.float32)
            nc.sync.dma_start(W2_t[:], W2_d[:])
            b1_t = cst.tile([H, 1], dt.float32)
            nc.sync.dma_start(b1_t[:], b1_d[:])
            b2_t = cst.tile([128, C], dt.float32)
            nc.sync.dma_start(b2_t[:], b2_d[:])
            io_t = cst.tile([128, 128], dt.float32)
            nc.sync.dma_start(io_t[:], io_d[:])
            ns_t = cst.tile([128, NBLK], dt.float32)
            nc.sync.dma_start(ns_t[:], ns_d[:])
            gi_t = res.tile([128, S_tot // 16], dt.int16)
            nc.sync.dma_start(gi_t[:], gi_d[:])
            gn_t = res.tile([128, S_tot // 128], dt.bfloat16)
            nc.sync.dma_start(gn_t[:], gn_d[:])
            gl_t = res.tile([128, S_tot // 128], dt.float32)
            nc.sync.dma_start(gl_t[:], gl_d[:])
            h01 = res.tile([128, NBLK, C], dt.bfloat16)
            agg = res.tile([128, NBLK, C], dt.float32)
            slf = [res.tile([128, NBLK, C], dt.float32, name=f"slf{i}") for i in range(2)]

            zsh = dram.tile([SHARD, C], dt.float32)
            # z replica, per (hop, window) tiles: window q holds slice q of
            # every core ([core][pos-within-slice] order).
            zwin = [
                [
                    dram.tile([w_sz[q], C], dt.float32, addr_space="Shared",
                              name=f"zw{k}_{q}")
                    for q in range(NQ)
                ]
                for k in range(k_hops)
            ]

            def fire_allgather(k, q):
                nc.gpsimd.collective_compute(
                    "AllGather", mybir.AluOpType.bypass, replica_groups=rg,
                    ins=[zsh[p_off[q] : p_off[q] + sz[q], :].opt()],
                    outs=[zwin[k][q][:].opt()],
                )

            # ---- MLP: h = relu(x @ W1 + b1) @ W2 + b2 ----
            with (
                tc.tile_pool(name="mlp", bufs=3) as mlp,
                tc.tile_pool(name="ps1", bufs=1, space="PSUM") as ps1,
                tc.tile_pool(name="ps2", bufs=1, space="PSUM") as ps2,
            ):
              for b in range(NBLK):
                xt = mlp.tile([128, KCH, 128], dt.bfloat16)
                for k in range(KCH):
                    nc.sync.dma_start(
                        xt[:, k, :], xT_d[k * 128 : (k + 1) * 128, b * 128 : (b + 1) * 128]
                    )
                p1 = ps1.tile([H, 128], dt.float32, space="PSUM")
                for k in range(KCH):
                    lw = W1_t[:, k * H : (k + 1) * H]
                    rw_ = xt[:, k, :]
                    if use_f32r:
                        lw = lw.bitcast(dt.float32r)
                        rw_ = rw_.bitcast(dt.float32r)
                    nc.tensor.matmul(
                        p1[:], lhsT=lw, rhs=rw_,
                        start=(k == 0), stop=(k == KCH - 1),
                    )
                rT = mlp.tile([H, 128], dt.float32)
                nc.scalar.activation(rT[:], p1[:], mybir.ActivationFunctionType.Relu, bias=b1_t[:])
                p2 = ps2.tile([128, C], dt.float32, space="PSUM")
                lw2, rw2 = rT[:], W2_t[:]
                if use_f32r:
                    lw2 = lw2.bitcast(dt.float32r)
                    rw2 = rw2.bitcast(dt.float32r)
                nc.tensor.matmul(p2[:], lhsT=lw2, rhs=rw2, start=True, stop=True)
                h = mlp.tile([128, C], dt.float32)
                nc.vector.tensor_tensor(h[:], p2[:], b2_t[:], op=mybir.AluOpType.add)
                nc.scalar.mul(h01[:, b, :], h[:], alpha)
                nc.vector.tensor_scalar(slf[0][:, b, :], h[:], ns_t[:, b : b + 1], None, op0=mybir.AluOpType.mult)
                nc.vector.tensor_tensor(slf[0][:, b, :], slf[0][:, b, :], h01[:, b, :], op=mybir.AluOpType.add)
                nc.sync.dma_start(zsh[b * 128 : (b + 1) * 128, :], h[:])
                for j in range(NQ):
                    if b == last_blk_of_slice[j]:
                        fire_allgather(0, j)

            def log_softmax_block(b):
                zb = agg[:, b, :]
                mx = sm.tile([128, 1], dt.float32)
                nc.vector.tensor_reduce(mx[:], zb, axis=mybir.AxisListType.X, op=mybir.AluOpType.max)
                zc = sm.tile([128, C], dt.float32)
                nc.vector.tensor_scalar(zc[:], zb, mx[:], None, op0=mybir.AluOpType.subtract)
                e = sm.tile([128, C], dt.float32)
                nc.scalar.activation(e[:], zc[:], mybir.ActivationFunctionType.Exp)
                sx = sm.tile([128, 1], dt.float32)
                nc.vector.tensor_reduce(sx[:], e[:], axis=mybir.AxisListType.X, op=mybir.AluOpType.add)
                ls = sm.tile([128, 1], dt.float32)
                nc.scalar.activation(ls[:], sx[:], mybir.ActivationFunctionType.Ln)
                o = sm.tile([128, C], dt.float32)
                nc.vector.tensor_scalar(o[:], zc[:], ls[:], None, op0=mybir.AluOpType.subtract)
                nc.sync.dma_start(out_d[b * 128 : (b + 1) * 128, :], o[:])

            # ---- K propagation hops ----
            for k in range(k_hops):
                cur, nxt = slf[k % 2], slf[(k + 1) % 2]
                last_hop = k == k_hops - 1
                for q in range(NQ):
                    a0 = int(chunk_off[q])
                    Lq = int(chunk_off[q + 1]) - a0
                    nbat = Lq // GB
                    zsrc = zwin[k][q][:]
                    # block schedule for this window: group -> block
                    blocks = np.repeat(np.arange(NBLK), gqb[q])
                    pcur = None
                    for bi in range(nbat):
                        s0 = a0 + bi * GB  # global slot offset
                        G = gw.tile([128, GPB, C], dt.float32)
                        nc.gpsimd.dma_gather(
                            out_ap=G[:],
                            in_ap=zsrc,
                            idxs_ap=gi_t[:, s0 // 16 : (s0 + GB) // 16],
                            num_idxs=GB,
                            num_idxs_reg=GB,
                            elem_size=C,
                            queue_num=(s0 // GB) % nqueues,
                        )
                        mmdt = dt.bfloat16 if use_bf16 else dt.float32
                        G2 = gw.tile([128, GPB, C], mmdt, name="G2", tag="G2")
                        nc.vector.tensor_tensor(
                            G2[:],
                            G[:],
                            gn_t[:, s0 // 128 : (s0 + GB) // 128].unsqueeze(2).to_broadcast([128, GPB, C]),
                            op=mybir.AluOpType.mult,
                        )
                        R = rw.tile([128, GPB, 128], mmdt)
                        nc.vector.tensor_tensor(
                            R[:],
                            io_t[:].unsqueeze(1).to_broadcast([128, GPB, 128]),
                            gl_t[:, s0 // 128 : (s0 + GB) // 128].unsqueeze(2).to_broadcast([128, GPB, 128]),
                            op=mybir.AluOpType.is_equal,
                        )
                        for j in range(GPB):
                            gg = bi * GPB + j  # group index within window
                            b = int(blocks[gg])
                            first = gg == 0 or int(blocks[gg - 1]) != b
                            last = gg == len(blocks) - 1 or int(blocks[gg + 1]) != b
                            if first:
                                pcur = psg.tile([128, C], dt.float32, space="PSUM", name="pg", tag="pg")
                            nc.tensor.matmul(
                                pcur[:], lhsT=R[:, j, :], rhs=G2[:, j, :],
                                start=first, stop=last,
                            )
                            if last:
                                if q == 0:
                                    nc.vector.tensor_tensor(
                                        agg[:, b, :], pcur[:],
                                        cur[:, b, :], op=mybir.AluOpType.add,
                                    )
                                else:
                                    nc.vector.tensor_tensor(
                                        agg[:, b, :],
                                        agg[:, b, :],
                                        pcur[:], op=mybir.AluOpType.add,
                                    )
                                if q == NQ - 1 and not last_hop:
                                    # block b final for this hop: write its z
                                    # rows; slice AllGathers fire early
                                    nc.sync.dma_start(
                                        zsh[b * 128 : (b + 1) * 128, :], agg[:, b, :]
                                    )
                                    for j2 in range(NQ):
                                        if b == last_blk_of_slice[j2]:
                                            fire_allgather(k + 1, j2)
                if not last_hop:
                    # stage the next hop's self/teleport term in bulk
                    nc.vector.tensor_tensor(
                        nxt[:], agg[:],
                        ns_t[:].unsqueeze(2).to_broadcast([128, NBLK, C]),
                        op=mybir.AluOpType.mult,
                    )
                    nc.vector.tensor_tensor(nxt[:], nxt[:], h01[:], op=mybir.AluOpType.add)

            # ---- log_softmax (after the gather storm; DVE is uncontended) ----
            for b in range(NBLK):
                log_softmax_block(b)


    nc.compile()
    res_ = run_bass_kernel_spmd(nc, in_maps, core_ids=list(range(ncores)), trace=trace)

    out = np.empty((N, C), np.float32)
    for c in range(ncores):
        nodes_c = prep["node_of"][c, : cnt_per_core[c]]
        out[nodes_c] = res_.results[c]["out"][: cnt_per_core[c]]
    return out, res_


# revision 13
# speedup vs baseline: 1.0840x; 1.0139x over previous
"""APPNP (MLP + K-hop personalized-propagation + log_softmax) on 8 TRN2 NeuronCores.

Strategy (graph/data parallel, per sharding hint):
- Nodes are sharded row-wise across the 8 cores with a greedy assignment that
  balances per-(source-window, dest-block) in-edge counts across cores (the
  schedule is common/SPMD, so per-bin max over cores sets the padding).
- W1/W2 are replicated.
- Per hop, each core processes the in-edges of its node shard:
  z rows are fetched from a replicated copy of z in HBM via gpsimd.dma_gather
  (edge-major, 256B rows), scaled by the gcn norm on DVE, and segment-summed
  per destination with one-hot matmuls on the TensorEngine (PSUM accumulate).
- Halo exchange: each core's z shard is split into 4 position-slices and the
  replicated z is stored slice-interleaved ([slice][core][pos]), so the
  exchange is 4 independent AllGathers per hop.  Slice j's AllGather fires as
  soon as the last dest block of slice j finishes its final (q=3) window, so
  the AllGather for window 0 of hop k+1 overlaps the tail of hop k and the
  next hop's gathers start with no inter-hop bubble.
- int16 gather indices limit one gather to a <=32768-row window of z; the
  slice-interleaved layout makes window q exactly slice q (<=25600 rows).
- Self-loops are not gathered; their contribution nsl[i]*z[i] is applied from
  the resident agg tile.
"""

import sys

sys.path.insert(0, "/opt/trn_rl_repo")

import numpy as np
import ml_dtypes


def kernel(x, edge_index, edge_weight, W1, b1, W2, b2):
    out, _ = appnp_trn(
        np.asarray(x, dtype=np.float32),
        np.asarray(edge_index, dtype=np.int32),
        np.asarray(edge_weight, dtype=np.float32),
        np.asarray(W1, dtype=np.float32),
        np.asarray(b1, dtype=np.float32),
        np.asarray(W2, dtype=np.float32),
        np.asarray(b2, dtype=np.float32),
        k_hops=10,
        alpha=0.1,
    )
    return out


def _slice_sizes(SHARD, nslices=4):
    """Per-core position-slice sizes, multiples of 128 (whole dest blocks)."""
    nblk = SHARD // 128
    base = nblk // nslices
    rem = nblk - base * nslices
    blks = [base + (1 if s < rem else 0) for s in range(nslices)]
    return [b * 128 for b in blks]


def _assign_cores(row_e, col_e, N, ncores, SHARD):
    """Assign nodes to (core, position).  Position comes from the in-degree
    order (strata of 8 similar-degree nodes); the greedy balances, per
    stratum, the per-source-window in-edge counts across cores.  The source
    window of a node is slice_of(pos), which is independent of the core
    assignment, so a single greedy pass suffices."""
    cnt = np.bincount(col_e, minlength=N)
    order = np.argsort(cnt, kind="stable")  # ascending in-degree
    nstrata = N // ncores
    strata = order.reshape(nstrata, ncores)
    pos_of = np.empty(N, np.int64)
    pos_of[order] = np.repeat(np.arange(nstrata), ncores)

    sz = _slice_sizes(SHARD)
    p_off = np.concatenate([[0], np.cumsum(sz)])
    slice_of_pos = np.searchsorted(p_off[1:], np.arange(SHARD), side="right")

    nq = len(sz)
    src_q = slice_of_pos[pos_of[row_e]]
    dq = np.zeros((N, nq), np.float64)
    np.add.at(dq, (col_e, src_q), 1.0)

    new_core = np.empty(N, np.int64)
    dqs = dq[strata]  # [nstrata, 8, nq]
    tot = dqs.sum(axis=2)
    ordd = np.argsort(-tot, axis=1)
    for b0 in range(0, nstrata, 128):
        running = np.zeros((ncores, nq))
        for rr in range(b0, min(b0 + 128, nstrata)):
            used = 0
            nodes = strata[rr]
            for k in ordd[rr]:
                n = nodes[k]
                best, bestv = -1, None
                cand = running + dq[n]
                mx = cand.max(axis=1) + 1e-3 * cand.sum(axis=1)
                for c in range(ncores):
                    if used >> c & 1:
                        continue
                    if best < 0 or mx[c] < bestv:
                        best, bestv = c, mx[c]
                new_core[n] = best
                used |= 1 << best
                running[best] += dq[n]
    return new_core, pos_of


def _host_prep(edge_index, edge_weight, N, ncores, alpha, SHARD, GB, cnt_per_core):
    """Build the permutation, shards and padded slot schedules."""
    NBLK = SHARD // 128

    sz = _slice_sizes(SHARD)  # per-core slice sizes
    nq = len(sz)
    p_off = np.concatenate([[0], np.cumsum(sz)])  # pos offsets within a core
    w_sz = [s * ncores for s in sz]  # window sizes in z-replica
    w_off = np.concatenate([[0], np.cumsum(w_sz)])
    assert max(w_sz) <= 32768

    # self-loops are NOT gathered: their contribution nsl[i]*z[i] is computed
    # on-chip from the resident agg tile.
    row_e = edge_index[0].astype(np.int64)
    col_e = edge_index[1].astype(np.int64)
    w_e = edge_weight.astype(np.float64)
    ar = np.arange(N, dtype=np.int64)
    deg = np.bincount(np.concatenate([col_e, ar]),
                      weights=np.concatenate([w_e, np.ones(N)]), minlength=N)
    dis = np.where(deg > 0, 1.0 / np.sqrt(deg), 0.0)
    norm = (dis[row_e] * w_e * dis[col_e] * (1.0 - alpha)).astype(np.float32)
    nsl = ((1.0 - alpha) * dis * dis).astype(np.float32)  # self-loop weight

    core_of, pos_of = _assign_cores(row_e, col_e, N, ncores, SHARD)
    node_of = np.full((ncores, SHARD), 0, np.int64)
    node_of[core_of, pos_of] = np.arange(N)

    slice_of_pos = np.searchsorted(p_off[1:], np.arange(SHARD), side="right")
    # z-replica row of a node: window base + core*slice_size + pos-within-slice
    sq_of = slice_of_pos[pos_of]
    zrow = w_off[sq_of] + core_of * np.array(sz)[sq_of] + (pos_of - p_off[sq_of])

    src_q = sq_of[row_e]
    src_rel = (zrow[row_e] - w_off[src_q]).astype(np.int64)
    dest_core = core_of[col_e]
    dest_pos = pos_of[col_e]

    per_core = []
    counts = np.zeros((ncores, nq * NBLK), np.int64)
    for c in range(ncores):
        m = dest_core == c
        dc = dest_pos[m]
        key = src_q[m] * NBLK + (dc >> 7)
        o = np.argsort(key, kind="stable")
        per_core.append(
            (
                key[o],
                src_rel[m][o],
                norm[m][o],
                (dc & 127).astype(np.float32)[o],
            )
        )
        counts[c] = np.bincount(key, minlength=nq * NBLK)

    # common cross-core schedule: groups (of 128 slots) per (window, block)
    gqb = -(-counts.max(axis=0) // 128)  # ceil
    gqb = gqb.reshape(nq, NBLK)
    # every (window, block) needs >=1 group: q=0 initializes the accumulator,
    # q=3 triggers the per-block zsh write / next-slf staging
    gqb = np.maximum(gqb, 1)
    # pad each window's slot count to a multiple of GB (gather batch)
    for q in range(nq):
        Lq = 128 * gqb[q].sum()
        pad = (-Lq) % GB
        gqb[q, NBLK - 1] += pad // 128
    gqb = gqb.reshape(-1)
    group_off = np.concatenate([[0], np.cumsum(128 * gqb)])
    S_tot = int(group_off[-1])
    gqb2 = gqb.reshape(nq, NBLK)
    Lq_arr = [int(128 * gqb2[q].sum()) for q in range(nq)]
    chunk_off = np.concatenate([[0], np.cumsum(Lq_arr)]).astype(np.int64)

    idx_w = np.zeros((ncores, 128, S_tot // 16), np.int16)
    nrm_t = np.zeros((ncores, 128, S_tot // 128), np.float32)
    lds_t = np.zeros((ncores, 128, S_tot // 128), np.float32)
    for c in range(ncores):
        ks, rels, nrms, ldss = per_core[c]
        first = np.concatenate([[0], np.cumsum(counts[c])[:-1]])
        rank = np.arange(ks.size) - first[ks]
        pos = group_off[ks] + rank
        rel16 = np.zeros(S_tot, np.int16)
        nrm_s = np.zeros(S_tot, np.float32)
        lds_s = np.zeros(S_tot, np.float32)
        rel16[pos] = rels.astype(np.int16)
        nrm_s[pos] = nrms
        lds_s[pos] = ldss
        for q in range(nq):
            a, b = int(chunk_off[q]), int(chunk_off[q + 1])
            idx_w[c, :, a // 16 : b // 16] = np.tile(
                rel16[a:b].reshape(-1, 16).T, (8, 1)
            )
            nrm_t[c, :, a // 128 : b // 128] = nrm_s[a:b].reshape(-1, 128).T
            lds_t[c, :, a // 128 : b // 128] = lds_s[a:b].reshape(-1, 128).T

    nsl_t = np.zeros((ncores, 128, NBLK), np.float32)
    for c in range(ncores):
        nodes_c = node_of[c, : cnt_per_core[c]]
        pos = np.arange(cnt_per_core[c])
        nsl_t[c, pos & 127, pos >> 7] = nsl[nodes_c]

    return dict(
        node_of=node_of,
        gqb=gqb2,
        chunk_off=chunk_off,
        S_tot=S_tot,
        idx_w=idx_w,
        nrm_t=nrm_t,
        lds_t=lds_t,
        nsl_t=nsl_t,
        sz=sz,
        p_off=p_off,
        w_sz=w_sz,
    )


def appnp_trn(
    x,
    edge_index,
    edge_weight,
    W1,
    b1,
    W2,
    b2,
    k_hops,
    alpha,
    GB=1024,
    ncores=8,
    trace=False,
    use_f32r=False,
    use_bf16=True,
    nqueues=4,
):
    from concourse import bass, bacc, tile, mybir, library_config
    from concourse.bass_utils import run_bass_kernel_spmd

    N, F = x.shape
    H = W1.shape[1]
    C = W2.shape[1]
    assert C == 64 and H <= 128 and GB % 128 == 0

    base = N // ncores
    cnt_per_core = np.full(ncores, base, np.int64)
    cnt_per_core[: N - base * ncores] += 1
    SHARD = -(-int(cnt_per_core.max()) // 128) * 128
    NBLK = SHARD // 128
    FP = -(-F // 128) * 128
    KCH = FP // 128
    NQ = 4

    prep = _host_prep(edge_index, edge_weight, N, ncores, alpha, SHARD, GB, cnt_per_core)
    gqb = prep["gqb"]
    chunk_off = prep["chunk_off"]
    S_tot = prep["S_tot"]
    sz = prep["sz"]          # per-core slice sizes (rows)
    p_off = prep["p_off"]    # per-core slice offsets (rows)
    w_sz = prep["w_sz"]      # z-replica window sizes (rows)
    GPB = GB // 128  # groups per gather batch

    blk_per_slice = [s // 128 for s in sz]
    blk_off = np.concatenate([[0], np.cumsum(blk_per_slice)])
    last_blk_of_slice = [int(blk_off[j + 1] - 1) for j in range(NQ)]
    slice_of_blk = np.searchsorted(blk_off[1:], np.arange(NBLK), side="right")

    # ---- per-core inputs ----
    in_maps = []
    W1p = np.zeros((FP, H), np.float32)
    W1p[:F] = W1
    b1c = b1.reshape(H, 1).astype(np.float32)
    b2r = np.tile(b2.reshape(1, C), (128, 1)).astype(np.float32)
    iota = np.tile(np.arange(128, dtype=np.float32), (128, 1))
    for c in range(ncores):
        xT = np.zeros((FP, SHARD), np.float32)
        nodes_c = prep["node_of"][c, : cnt_per_core[c]]
        xT[:F, : cnt_per_core[c]] = x[nodes_c].T
        in_maps.append(
            {
                "xT": xT,
                "W1": W1p,
                "b1": b1c,
                "W2": W2.astype(np.float32),
                "b2r": b2r,
                "iota": iota,
                "nsl": prep["nsl_t"][c],
                "gidx": prep["idx_w"][c],
                "gnrm": prep["nrm_t"][c].astype(ml_dtypes.bfloat16),
                "glds": prep["lds_t"][c],
            }
        )

    # ---- build the device program ----
    nc = bacc.Bacc(
        "TRN2", target_bir_lowering=False, debug=False, num_devices=ncores,
        num_swdge_queues=nqueues,
    )
    dt = mybir.dt
    xT_d = nc.dram_tensor("xT", [FP, SHARD], dt.float32, kind="ExternalInput").ap()
    W1_d = nc.dram_tensor("W1", [FP, H], dt.float32, kind="ExternalInput").ap()
    b1_d = nc.dram_tensor("b1", [H, 1], dt.float32, kind="ExternalInput").ap()
    W2_d = nc.dram_tensor("W2", [H, C], dt.float32, kind="ExternalInput").ap()
    b2_d = nc.dram_tensor("b2r", [128, C], dt.float32, kind="ExternalInput").ap()
    io_d = nc.dram_tensor("iota", [128, 128], dt.float32, kind="ExternalInput").ap()
    ns_d = nc.dram_tensor("nsl", [128, SHARD // 128], dt.float32, kind="ExternalInput").ap()
    gi_d = nc.dram_tensor("gidx", [128, S_tot // 16], dt.int16, kind="ExternalInput").ap()
    gn_d = nc.dram_tensor("gnrm", [128, S_tot // 128], dt.bfloat16, kind="ExternalInput").ap()
    gl_d = nc.dram_tensor("glds", [128, S_tot // 128], dt.float32, kind="ExternalInput").ap()
    out_d = nc.dram_tensor("out", [SHARD, C], dt.float32, kind="ExternalOutput").ap()

    rg = [list(range(ncores))]

    with tile.TileContext(nc) as tc:
        nc.gpsimd.load_library(library_config.mlp)
        with (
            tc.tile_pool(name="const", bufs=1) as cst,
            tc.tile_pool(name="resident", bufs=1) as res,
            tc.tile_pool(name="gw", bufs=8) as gw,
            tc.tile_pool(name="rw", bufs=6) as rw,
            tc.tile_pool(name="sm", bufs=2) as sm,
            tc.tile_pool(name="psg", bufs=6, space="PSUM") as psg,
            tc.tile_pool(name="dram", bufs=1, space="DRAM") as dram,
        ):
            # constants / resident data
            W1_t = cst.tile([128, KCH * H], dt.float32)
            for k in range(KCH):
                nc.sync.dma_start(W1_t[:, k * H : (k + 1) * H], W1_d[k * 128 : (k + 1) * 128, :])
            W2_t = cst.tile([H, C], dt.float32)
            nc.sync.dma_start(W2_t[:], W2_d[:])
            b1_t = cst.tile([H, 1], dt.float32)
            nc.sync.dma_start(b1_t[:], b1_d[:])
            b2_t = cst.tile([128, C], dt.float32)
            nc.sync.dma_start(b2_t[:], b2_d[:])
            io_t = cst.tile([128, 128], dt.float32)
            nc.sync.dma_start(io_t[:], io_d[:])
            ns_t = cst.tile([128, NBLK], dt.float32)
            nc.sync.dma_start(ns_t[:], ns_d[:])
            gi_t = res.tile([128, S_tot // 16], dt.int16)
            nc.sync.dma_start(gi_t[:], gi_d[:])
            gn_t = res.tile([128, S_tot // 128], dt.bfloat16)
            nc.sync.dma_start(gn_t[:], gn_d[:])
            gl_t = res.tile([128, S_tot // 128], dt.float32)
            nc.sync.dma_start(gl_t[:], gl_d[:])
            h01 = res.tile([128, NBLK, C], dt.bfloat16)
            agg = res.tile([128, NBLK, C], dt.float32)
            slf = [res.tile([128, NBLK, C], dt.float32, name=f"slf{i}") for i in range(2)]

            zsh = dram.tile([SHARD, C], dt.float32)
            # z replica, per (hop, window) tiles: window q holds slice q of
            # every core ([core][pos-within-slice] order).
            zwin = [
                [
                    dram.tile([w_sz[q], C], dt.float32, addr_space="Shared",
                              name=f"zw{k}_{q}")
                    for q in range(NQ)
                ]
                for k in range(k_hops)
            ]

            def fire_allgather(k, q):
                nc.gpsimd.collective_compute(
                    "AllGather", mybir.AluOpType.bypass, replica_groups=rg,
                    ins=[zsh[p_off[q] : p_off[q] + sz[q], :].opt()],
                    outs=[zwin[k][q][:].opt()],
                )

            # ---- MLP: h = relu(x @ W1 + b1) @ W2 + b2 ----
            with (
                tc.tile_pool(name="mlp", bufs=3) as mlp,
                tc.tile_pool(name="ps1", bufs=1, space="PSUM") as ps1,
                tc.tile_pool(name="ps2", bufs=1, space="PSUM") as ps2,
            ):
              for b in range(NBLK):
                xt = mlp.tile([128, KCH, 128], dt.float32)
                for k in range(KCH):
                    nc.sync.dma_start(
                        xt[:, k, :], xT_d[k * 128 : (k + 1) * 128, b * 128 : (b + 1) * 128]
                    )
                p1 = ps1.tile([H, 128], dt.float32, space="PSUM")
                for k in range(KCH):
                    lw = W1_t[:, k * H : (k + 1) * H]
                    rw_ = xt[:, k, :]
                    if use_f32r:
                        lw = lw.bitcast(dt.float32r)
                        rw_ = rw_.bitcast(dt.float32r)
                    nc.tensor.matmul(
                        p1[:], lhsT=lw, rhs=rw_,
                        start=(k == 0), stop=(k == KCH - 1),
                    )
                rT = mlp.tile([H, 128], dt.float32)
                nc.scalar.activation(rT[:], p1[:], mybir.ActivationFunctionType.Relu, bias=b1_t[:])
                p2 = ps2.tile([128, C], dt.float32, space="PSUM")
                lw2, rw2 = rT[:], W2_t[:]
                if use_f32r:
                    lw2 = lw2.bitcast(dt.float32r)
                    rw2 = rw2.bitcast(dt.float32r)
                nc.tensor.matmul(p2[:], lhsT=lw2, rhs=rw2, start=True, stop=True)
                h = mlp.tile([128, C], dt.float32)
                nc.vector.tensor_tensor(h[:], p2[:], b2_t[:], op=mybir.AluOpType.add)
                nc.scalar.mul(h01[:, b, :], h[:], alpha)
                nc.vector.tensor_scalar(slf[0][:, b, :], h[:], ns_t[:, b : b + 1], None, op0=mybir.AluOpType.mult)
                nc.vector.tensor_tensor(slf[0][:, b, :], slf[0][:, b, :], h01[:, b, :], op=mybir.AluOpType.add)
                nc.sync.dma_start(zsh[b * 128 : (b + 1) * 128, :], h[:])
                for j in range(NQ):
                    if b == last_blk_of_slice[j]:
                        fire_allgather(0, j)

            def log_softmax_block(b):
                zb = agg[:, b, :]
                mx = sm.tile([128, 1], dt.float32)
                nc.vector.tensor_reduce(mx[:], zb, axis=mybir.AxisListType.X, op=mybir.AluOpType.max)
                zc = sm.tile([128, C], dt.float32)
                nc.vector.tensor_scalar(zc[:], zb, mx[:], None, op0=mybir.AluOpType.subtract)
                e = sm.tile([128, C], dt.float32)
                nc.scalar.activation(e[:], zc[:], mybir.ActivationFunctionType.Exp)
                sx = sm.tile([128, 1], dt.float32)
                nc.vector.tensor_reduce(sx[:], e[:], axis=mybir.AxisListType.X, op=mybir.AluOpType.add)
                ls = sm.tile([128, 1], dt.float32)
                nc.scalar.activation(ls[:], sx[:], mybir.ActivationFunctionType.Ln)
                o = sm.tile([128, C], dt.float32)
                nc.vector.tensor_scalar(o[:], zc[:], ls[:], None, op0=mybir.AluOpType.subtract)
                nc.sync.dma_start(out_d[b * 128 : (b + 1) * 128, :], o[:])

            # ---- K propagation hops ----
            for k in range(k_hops):
                cur, nxt = slf[k % 2], slf[(k + 1) % 2]
                last_hop = k == k_hops - 1
                for q in range(NQ):
                    a0 = int(chunk_off[q])
                    Lq = int(chunk_off[q + 1]) - a0
                    nbat = Lq // GB
                    zsrc = zwin[k][q][:]
                    # block schedule for this window: group -> block
                    blocks = np.repeat(np.arange(NBLK), gqb[q])
                    pcur = None
                    for bi in range(nbat):
                        s0 = a0 + bi * GB  # global slot offset
                        G = gw.tile([128, GPB, C], dt.float32)
                        nc.gpsimd.dma_gather(
                            out_ap=G[:],
                            in_ap=zsrc,
                            idxs_ap=gi_t[:, s0 // 16 : (s0 + GB) // 16],
                            num_idxs=GB,
                            num_idxs_reg=GB,
                            elem_size=C,
                            queue_num=(s0 // GB) % nqueues,
                        )
                        mmdt = dt.bfloat16 if use_bf16 else dt.float32
                        G2 = gw.tile([128, GPB, C], mmdt, name="G2", tag="G2")
                        nc.vector.tensor_tensor(
                            G2[:],
                            G[:],
                            gn_t[:, s0 // 128 : (s0 + GB) // 128].unsqueeze(2).to_broadcast([128, GPB, C]),
                            op=mybir.AluOpType.mult,
                        )
                        R = rw.tile([128, GPB, 128], mmdt)
                        nc.vector.tensor_tensor(
                            R[:],
                            io_t[:].unsqueeze(1).to_broadcast([128, GPB, 128]),
                            gl_t[:, s0 // 128 : (s0 + GB) // 128].unsqueeze(2).to_broadcast([128, GPB, 128]),
                            op=mybir.AluOpType.is_equal,
                        )
                        for j in range(GPB):
                            gg = bi * GPB + j  # group index within window
                            b = int(blocks[gg])
                            first = gg == 0 or int(blocks[gg - 1]) != b
                            last = gg == len(blocks) - 1 or int(blocks[gg + 1]) != b
                            if first:
                                pcur = psg.tile([128, C], dt.float32, space="PSUM", name="pg", tag="pg")
                            nc.tensor.matmul(
                                pcur[:], lhsT=R[:, j, :], rhs=G2[:, j, :],
                                start=first, stop=last,
                            )
                            if last:
                                if q == 0:
                                    nc.vector.tensor_tensor(
                                        agg[:, b, :], pcur[:],
                                        cur[:, b, :], op=mybir.AluOpType.add,
                                    )
                                else:
                                    nc.vector.tensor_tensor(
                                        agg[:, b, :],
                                        agg[:, b, :],
                                        pcur[:], op=mybir.AluOpType.add,
                                    )
                                if q == NQ - 1 and not last_hop:
                                    # block b final for this hop: write its z
                                    # rows; slice AllGathers fire early
                                    nc.sync.dma_start(
                                        zsh[b * 128 : (b + 1) * 128, :], agg[:, b, :]
                                    )
                                    for j2 in range(NQ):
                                        if b == last_blk_of_slice[j2]:
                                            fire_allgather(k + 1, j2)
                if not last_hop:
                    # stage the next hop's self/teleport term in bulk
                    nc.vector.tensor_tensor(
                        nxt[:], agg[:],
                        ns_t[:].unsqueeze(2).to_broadcast([128, NBLK, C]),
                        op=mybir.AluOpType.mult,
                    )
                    nc.vector.tensor_tensor(nxt[:], nxt[:], h01[:], op=mybir.AluOpType.add)

            # ---- log_softmax (after the gather storm; DVE is uncontended) ----
            for b in range(NBLK):
                log_softmax_block(b)


    nc.compile()
    res_ = run_bass_kernel_spmd(nc, in_maps, core_ids=list(range(ncores)), trace=trace)

    out = np.empty((N, C), np.float32)
    for c in range(ncores):
        nodes_c = prep["node_of"][c, : cnt_per_core[c]]
        out[nodes_c] = res_.results[c]["out"][: cnt_per_core[c]]
    return out, res_
